# revision 10
# baseline (speedup 1.0000x reference)
"""MultiQueryAttention Trainium2 kernel (8 NeuronCores, SPMD).

Reference computation (per batch b):
    q_proj = q @ Wq            [T, C] -> [T, H, D]   (H=16 heads, D=64)
    k_proj = k @ Wk            [T, D]   (single shared KV head)
    v_proj = v @ Wv            [T, D]
    S_h    = q_h @ k_proj.T / sqrt(D)      [T, T] per head
    P      = softmax(S)        (no mask)
    out    = (P @ v_proj  for each head) -> [T, C]; out @ Wp + bp

Sharding: 8 cores = batch (4) x head-halves (2). Each core handles one
batch and 8 query heads; the shared K/V projections are replicated.
Wq is split column-wise, Wp row-wise; each pair of cores produces a
partial [T, C] output that the host sums (+ bp).

Device layout notes:
  - All matmul operands are bf16 (PE streams bf16 at 1 cyc/row vs 2 for
    fp32); PSUM accumulation is fp32.
  - Host pre-transposes q/k/v to [C, T] so every projection contraction
    (over C) has C on the partition axis.
  - Scores are computed transposed: S^T[tk, tq] so that P^T can feed the
    P@V matmul directly as the stationary operand.  The two heads of a
    head-pair run concurrently in the PE array via row tiling (K=64 each,
    base partitions 0 and 64).
  - Row-sums of P come for free from a ones-column appended to v_proj
    (stationary [v | 1] -> output row 64 is the softmax denominator).
  - softmax(x) is computed without max-subtraction: scores are ~N(0, 0.4)
    here so exp is safe in fp32, and the reference's max-subtraction is
    mathematically a no-op.
"""

import numpy as np
import ml_dtypes
from contextlib import ExitStack

import concourse.bacc as bacc
import concourse.bass as bass
import concourse.mybir as mybir
import concourse.tile as tile

B, T, C = 4, 2048, 1024
H, D = 16, 64
HPC = 8              # heads per core
HD = HPC * D         # 512 per-core attention output dims
NCORES = 8
P128 = 128
NCC = C // P128      # 8 contraction chunks over C
NTK = T // P128      # 16 key chunks
NTQB = 4             # query blocks of 512
TQB = 512
NTP = 4              # head-pairs per core
SCALE = 1.0 / 8.0    # 1/sqrt(64)

BF = mybir.dt.bfloat16
F32 = mybir.dt.float32
NPBF = ml_dtypes.bfloat16


def emit_kernel(ctx: ExitStack, tc: tile.TileContext, dr):
    nc = tc.nc
    EXP = mybir.ActivationFunctionType.Exp

    const = ctx.enter_context(tc.tile_pool(name="const", bufs=1))
    persist = ctx.enter_context(tc.tile_pool(name="persist", bufs=1))
    stream = ctx.enter_context(tc.tile_pool(name="stream", bufs=2))
    ppool = ctx.enter_context(tc.tile_pool(name="ppool", bufs=3))
    small = ctx.enter_context(tc.tile_pool(name="small", bufs=2))
    outp = ctx.enter_context(tc.tile_pool(name="outp", bufs=3))
    # PSUM budget (8 banks): s2 rotation 2x2 + pv accum 2 + wp/vproj 2
    ps_s2 = ctx.enter_context(tc.tile_pool(name="ps_s2", bufs=2, space="PSUM"))
    ps_pv = ctx.enter_context(tc.tile_pool(name="ps_pv", bufs=1, space="PSUM"))
    ps_wp = ctx.enter_context(tc.tile_pool(name="ps_wp", bufs=1, space="PSUM"))
    dram = ctx.enter_context(tc.tile_pool(name="dram", bufs=2, space="DRAM"))

    # ---- weights to SBUF ----
    wq_sb = const.tile([P128, NCC, HD], BF)          # [c-in-chunk, cc, dcol]
    nc.sync.dma_start(wq_sb, dr["wq"].ap().rearrange("(cc p) d -> p cc d", p=P128))
    wk2_sb = const.tile([P128, NCC, P128], BF)       # Wk duplicated -> [*, 128]
    nc.sync.dma_start(wk2_sb, dr["wk2"].ap().rearrange("(cc p) d -> p cc d", p=P128))
    wv_sb = const.tile([P128, NCC, D], BF)
    nc.sync.dma_start(wv_sb, dr["wv"].ap().rearrange("(cc p) d -> p cc d", p=P128))
    wp_sb = const.tile([P128, HD // P128, C], BF)    # [hd-in-chunk, r, c-out]
    nc.sync.dma_start(wp_sb, dr["wp"].ap().rearrange("(r p) c -> p r c", p=P128))

    # qT resident (bf16, 4MB)
    qt_sb = persist.tile([P128, NCC, T], BF)
    nc.sync.dma_start(qt_sb, dr["qT"].ap().rearrange("(cc p) t -> p cc t", p=P128))

    # ---- K projection: k2[0:64]=k_projT, k2[64:128]=k_projT (dup) ----
    k2_sb = persist.tile([P128, T], BF)
    kps = [ps_s2.tile([P128, 1024], F32, tag="ps_s2", name=f"kps{i}") for i in range(2)]
    for cc in range(NCC):
        kt = stream.tile([P128, T], BF, tag="kv_chunk")
        nc.sync.dma_start(kt, dr["kT"].ap().rearrange("(cc p) t -> p cc t", p=P128)[:, cc, :])
        for blk in range(4):
            nc.tensor.matmul(
                kps[blk // 2][:, (blk % 2) * 512:(blk % 2) * 512 + 512],
                wk2_sb[:, cc, :], kt[:, blk * 512:(blk + 1) * 512],
                start=(cc == 0), stop=(cc == NCC - 1))
    nc.vector.tensor_copy(k2_sb[:, 0:1024], kps[0])
    nc.vector.tensor_copy(k2_sb[:, 1024:2048], kps[1])

    # ---- V projection (natural layout) + ones column ----
    # one accumulation chain per PSUM bank: t-tile outer, cc inner
    v65_sb = persist.tile([P128, NTK, D + 1], BF)
    vt_sb = stream.tile([P128, NCC, T], BF, tag="vt_all", bufs=1)
    nc.sync.dma_start(vt_sb, dr["vT"].ap().rearrange("(cc p) t -> p cc t", p=P128))
    for half in range(2):
        vps = ps_wp.tile([P128, 1024], F32, tag="ps_wp", name=f"vps{half}")
        for tk8 in range(8):
            tk = half * 8 + tk8
            for cc in range(NCC):
                nc.tensor.matmul(
                    vps[:, tk8 * D:(tk8 + 1) * D],
                    vt_sb[:, cc, tk * P128:(tk + 1) * P128], wv_sb[:, cc, :],
                    start=(cc == 0), stop=(cc == NCC - 1))
        nc.vector.tensor_copy(
            v65_sb[:, half * 8:(half + 1) * 8, 0:D],
            vps[:, 0:512].rearrange("p (tk d) -> p tk d", tk=8))
    nc.vector.memset(v65_sb[:, :, D:D + 1], 1.0)

    # ---- Q projection, per (dcol j, tq block): interleaved with attention ----
    qpt_sb = persist.tile([P128, NTP, T], BF)

    def qproj_chain(j, tqb):
        qps = ps_s2.tile([P128, 1024], F32, tag="ps_s2", name=f"qps_{j}_{tqb}")
        for cc in range(NCC):
            nc.tensor.matmul(
                qps[:, 0:512],
                wq_sb[:, cc, j * P128:(j + 1) * P128],
                qt_sb[:, cc, tqb * 512:(tqb + 1) * 512],
                start=(cc == 0), stop=(cc == NCC - 1))
        nc.vector.tensor_copy(
            qpt_sb[:, j, tqb * 512:(tqb + 1) * 512], qps[:, 0:512])

    attn_sb = persist.tile([P128, NTP, T], BF)   # attn_outT (normalized), bf16

    def attn_block(t, tqb):
        tq0 = tqb * TQB
        pv = ps_pv.tile([P128, 1024], F32, tag="ps_pv", name=f"pv_{t}_{tqb}")
        for c in range(NTK):
            s2 = ps_s2.tile([P128, 1024], F32, tag="ps_s2", name=f"s2_{t}_{tqb}_{c}")
            # head pair via PE row tiling (K=64 at base partitions 0 / 64)
            nc.tensor.matmul(
                s2[:, 0:512],
                k2_sb[0:64, c * P128:(c + 1) * P128],
                qpt_sb[0:64, t, tq0:tq0 + TQB],
                start=True, stop=True)
            nc.tensor.matmul(
                s2[:, 512:1024],
                k2_sb[64:128, c * P128:(c + 1) * P128],
                qpt_sb[64:128, t, tq0:tq0 + TQB],
                start=True, stop=True)
            p = ppool.tile([P128, 1024], BF, tag="p", name=f"p_{t}_{tqb}_{c}")
            nc.scalar.activation(p, s2, EXP, scale=SCALE)
            nc.tensor.matmul(
                pv[0:65, 0:512], v65_sb[:, c, :], p[:, 0:512],
                start=(c == 0), stop=(c == NTK - 1))
            nc.tensor.matmul(
                pv[0:65, 512:1024], v65_sb[:, c, :], p[:, 512:1024],
                start=(c == 0), stop=(c == NTK - 1))
        # normalize: rows 0..63 / row 64 (per tq, per head)
        ss = small.tile([1, 1024], F32, tag="ss", name=f"ss_{t}_{tqb}")
        nc.vector.tensor_copy(ss, pv[64:65, :])   # custom-DVE op below
        r = small.tile([1, 1024], F32, tag="r", name=f"r_{t}_{tqb}")
        nc.vector.reciprocal_approx_fast(out=r, in_=ss)
        # partition-broadcast r across 64 lanes (bounce via DRAM: DMA
        # cannot zero-step an SBUF source partition)
        rd = dram.tile([1, 1024], F32, tag="rd", name=f"rd_{t}_{tqb}")
        nc.sync.dma_start(rd, r)
        rb = small.tile([64, 1024], F32, tag="rb", name=f"rb_{t}_{tqb}")
        nc.sync.dma_start(rb, rd.to_broadcast([64, 1024]))
        nc.vector.tensor_mul(
            attn_sb[0:64, t, tq0:tq0 + TQB], pv[0:64, 0:512], rb[:, 0:512])
        h2s = small.tile([64, 512], BF, tag="h2s", name=f"h2s_{t}_{tqb}")
        nc.vector.tensor_mul(h2s, pv[0:64, 512:1024], rb[:, 512:1024])
        nc.sync.dma_start(attn_sb[64:128, t, tq0:tq0 + TQB], h2s)

    def wp_tile(tt):
        po = ps_wp.tile([P128, 1024], F32, tag="ps_wp", name=f"po_{tt}")
        for rr in range(HD // P128):
            lhsT = attn_sb[:, rr, tt * P128:(tt + 1) * P128]
            nc.tensor.matmul(po[:, 0:512], lhsT, wp_sb[:, rr, 0:512],
                             start=(rr == 0), stop=(rr == 3))
            nc.tensor.matmul(po[:, 512:1024], lhsT, wp_sb[:, rr, 512:1024],
                             start=(rr == 0), stop=(rr == 3))
        os_ = outp.tile([P128, 1024], F32, tag="os", name=f"os_{tt}")
        nc.vector.tensor_copy(os_, po)
        nc.sync.dma_start(dr["out"].ap()[tt * P128:(tt + 1) * P128, :], os_)

    # schedule: tqb-outer so Wp(tqb) overlaps the next tqb's attention;
    # q-proj chains for tqb+1 interleave into tqb's ACT-bound stream.
    for j in range(NTP):
        qproj_chain(j, 0)
    for tqb in range(NTQB):
        for t in range(NTP):
            attn_block(t, tqb)
            if tqb < NTQB - 1:
                qproj_chain(t, tqb + 1)
        for tt in range(4 * tqb, 4 * tqb + 4):
            wp_tile(tt)


def build_nc():
    nc = bacc.Bacc("TRN2", target_bir_lowering=False, debug=False)
    dr = {
        "qT": nc.dram_tensor("qT", [C, T], BF, kind="ExternalInput"),
        "kT": nc.dram_tensor("kT", [C, T], BF, kind="ExternalInput"),
        "vT": nc.dram_tensor("vT", [C, T], BF, kind="ExternalInput"),
        "wq": nc.dram_tensor("wq", [C, HD], BF, kind="ExternalInput"),
        "wk2": nc.dram_tensor("wk2", [C, P128], BF, kind="ExternalInput"),
        "wv": nc.dram_tensor("wv", [C, D], BF, kind="ExternalInput"),
        "wp": nc.dram_tensor("wp", [HD, C], BF, kind="ExternalInput"),
        "out": nc.dram_tensor("out", [T, C], F32, kind="ExternalOutput"),
    }
    with tile.TileContext(nc) as tc, ExitStack() as ctx:
        emit_kernel(ctx, tc, dr)
    nc.compile()
    return nc


_NC_CACHE = None


def _get_nc():
    global _NC_CACHE
    if _NC_CACHE is None:
        _NC_CACHE = build_nc()
    return _NC_CACHE


def make_in_maps(q, k, v, Wq, Wk, Wv, Wp):
    """Per-core input dicts (host-side sharding + transpose + bf16 cast)."""
    bf = lambda x: np.ascontiguousarray(x).astype(NPBF)
    wk2 = np.concatenate([Wk, Wk], axis=1)
    per_b = []
    for b in range(B):
        per_b.append((bf(q[b].T), bf(k[b].T), bf(v[b].T)))
    in_maps = []
    for core in range(NCORES):
        b, g = core // 2, core % 2
        qT, kT, vT = per_b[b]
        in_maps.append({
            "qT": qT, "kT": kT, "vT": vT,
            "wq": bf(Wq[:, g * HD:(g + 1) * HD]),
            "wk2": bf(wk2),
            "wv": bf(Wv),
            "wp": bf(Wp[g * HD:(g + 1) * HD, :]),
        })
    return in_maps


def kernel(q, k, v, Wq, Wk, Wv, Wp, bp):
    from concourse.bass_utils import run_bass_kernel_spmd

    q, k, v, Wq, Wk, Wv, Wp, bp = (np.asarray(x, np.float32)
                                   for x in (q, k, v, Wq, Wk, Wv, Wp, bp))
    nc = _get_nc()
    in_maps = make_in_maps(q, k, v, Wq, Wk, Wv, Wp)
    res = run_bass_kernel_spmd(nc, in_maps, list(range(NCORES))).results
    out = np.empty((B, T, C), np.float32)
    for b in range(B):
        out[b] = res[2 * b]["out"] + res[2 * b + 1]["out"] + bp
    return out


# revision 12
# speedup vs baseline: 1.2695x; 1.2695x over previous
"""MultiQueryAttention Trainium2 kernel (8 NeuronCores, SPMD).

Reference computation (per batch b):
    q_proj = q @ Wq            [T, C] -> [T, H, D]   (H=16 heads, D=64)
    k_proj = k @ Wk            [T, D]   (single shared KV head)
    v_proj = v @ Wv            [T, D]
    S_h    = q_h @ k_proj.T / sqrt(D)      [T, T] per head
    P      = softmax(S)        (no mask)
    out    = (P @ v_proj  for each head) -> [T, C]; out @ Wp + bp

Sharding: 8 cores = batch (4) x head-halves (2). Each core handles one
batch and 8 query heads; the shared K/V projections are replicated.
Wq is split column-wise, Wp row-wise; each pair of cores produces a
partial [T, C] output that the host sums (+ bp).

Device layout notes:
  - All matmul operands are bf16 (PE streams bf16 at 1 cyc/row vs 2 for
    fp32); PSUM accumulation is fp32.
  - Host pre-transposes q/k/v to [C, T] so every projection contraction
    (over C) has C on the partition axis.
  - Scores are computed transposed: S^T[tk, tq] so that P^T can feed the
    P@V matmul directly as the stationary operand.  The two heads of a
    head-pair run concurrently in the PE array via row tiling (K=64 each,
    base partitions 0 and 64).
  - Row-sums of P come for free from a ones-column appended to v_proj
    (stationary [v | 1] -> output row 64 is the softmax denominator).
  - softmax(x) is computed without max-subtraction: scores are ~N(0, 0.4)
    here so exp is safe in fp32, and the reference's max-subtraction is
    mathematically a no-op.
"""

import numpy as np
import ml_dtypes
from contextlib import ExitStack

import concourse.bacc as bacc
import concourse.bass as bass
import concourse.mybir as mybir
import concourse.tile as tile

B, T, C = 4, 2048, 1024
H, D = 16, 64
HPC = 8              # heads per core
HD = HPC * D         # 512 per-core attention output dims
NCORES = 8
P128 = 128
NCC = C // P128      # 8 contraction chunks over C
NTK = T // P128      # 16 key chunks
NTQB = 4             # query blocks of 512
TQB = 512
NTP = 4              # head-pairs per core
SCALE = 1.0 / 8.0    # 1/sqrt(64)

BF = mybir.dt.bfloat16
F32 = mybir.dt.float32
NPBF = ml_dtypes.bfloat16


def emit_kernel(ctx: ExitStack, tc: tile.TileContext, dr):
    nc = tc.nc
    EXP = mybir.ActivationFunctionType.Exp

    const = ctx.enter_context(tc.tile_pool(name="const", bufs=1))
    persist = ctx.enter_context(tc.tile_pool(name="persist", bufs=1))
    stream = ctx.enter_context(tc.tile_pool(name="stream", bufs=2))
    ppool = ctx.enter_context(tc.tile_pool(name="ppool", bufs=3))
    small = ctx.enter_context(tc.tile_pool(name="small", bufs=2))
    outp = ctx.enter_context(tc.tile_pool(name="outp", bufs=3))
    # PSUM budget (8 banks): s2 rotation 2x2 + pv accum 2 + wp/vproj 2
    ps_s2 = ctx.enter_context(tc.tile_pool(name="ps_s2", bufs=2, space="PSUM"))
    ps_pv = ctx.enter_context(tc.tile_pool(name="ps_pv", bufs=1, space="PSUM"))
    ps_wp = ctx.enter_context(tc.tile_pool(name="ps_wp", bufs=1, space="PSUM"))
    dram = ctx.enter_context(tc.tile_pool(name="dram", bufs=2, space="DRAM"))

    # ---- weights to SBUF ----
    wq_sb = const.tile([P128, NCC, HD], BF)          # [c-in-chunk, cc, dcol]
    nc.sync.dma_start(wq_sb, dr["wq"].ap().rearrange("(cc p) d -> p cc d", p=P128))
    wk2_sb = const.tile([P128, NCC, P128], BF)       # Wk duplicated -> [*, 128]
    nc.sync.dma_start(wk2_sb, dr["wk2"].ap().rearrange("(cc p) d -> p cc d", p=P128))
    wv_sb = const.tile([P128, NCC, D], BF)
    nc.sync.dma_start(wv_sb, dr["wv"].ap().rearrange("(cc p) d -> p cc d", p=P128))
    wp_sb = const.tile([P128, HD // P128, C], BF)    # [hd-in-chunk, r, c-out]
    nc.sync.dma_start(wp_sb, dr["wp"].ap().rearrange("(r p) c -> p r c", p=P128))

    # qT resident (bf16, 4MB)
    qt_sb = persist.tile([P128, NCC, T], BF)
    nc.sync.dma_start(qt_sb, dr["qT"].ap().rearrange("(cc p) t -> p cc t", p=P128))

    # ---- K projection: k2[0:64]=k_projT, k2[64:128]=k_projT (dup) ----
    k2_sb = persist.tile([P128, T], BF)
    kps = [ps_s2.tile([P128, 1024], F32, tag="ps_s2", name=f"kps{i}") for i in range(2)]
    for cc in range(NCC):
        kt = stream.tile([P128, T], BF, tag="kv_chunk")
        nc.sync.dma_start(kt, dr["kT"].ap().rearrange("(cc p) t -> p cc t", p=P128)[:, cc, :])
        for blk in range(4):
            nc.tensor.matmul(
                kps[blk // 2][:, (blk % 2) * 512:(blk % 2) * 512 + 512],
                wk2_sb[:, cc, :], kt[:, blk * 512:(blk + 1) * 512],
                start=(cc == 0), stop=(cc == NCC - 1))
    nc.vector.tensor_copy(k2_sb[:, 0:1024], kps[0])
    nc.vector.tensor_copy(k2_sb[:, 1024:2048], kps[1])

    # ---- V projection (natural layout) + ones column ----
    # one accumulation chain per PSUM bank: t-tile outer, cc inner
    v65_sb = persist.tile([P128, NTK, D + 1], BF)
    vt_sb = stream.tile([P128, NCC, T], BF, tag="vt_all", bufs=1)
    nc.sync.dma_start(vt_sb, dr["vT"].ap().rearrange("(cc p) t -> p cc t", p=P128))
    for half in range(2):
        vps = ps_wp.tile([P128, 1024], F32, tag="ps_wp", name=f"vps{half}")
        for tk8 in range(8):
            tk = half * 8 + tk8
            for cc in range(NCC):
                nc.tensor.matmul(
                    vps[:, tk8 * D:(tk8 + 1) * D],
                    vt_sb[:, cc, tk * P128:(tk + 1) * P128], wv_sb[:, cc, :],
                    start=(cc == 0), stop=(cc == NCC - 1))
        nc.vector.tensor_copy(
            v65_sb[:, half * 8:(half + 1) * 8, 0:D],
            vps[:, 0:512].rearrange("p (tk d) -> p tk d", tk=8))
    nc.vector.memset(v65_sb[:, :, D:D + 1], 1.0)

    # ---- Q projection, per (dcol j, tq block): interleaved with attention ----
    qpt_sb = persist.tile([P128, NTP, T], BF)

    def qproj_chain(j, tqb):
        qps = ps_s2.tile([P128, 1024], F32, tag="ps_s2", name=f"qps_{j}_{tqb}")
        for cc in range(NCC):
            nc.tensor.matmul(
                qps[:, 0:512],
                wq_sb[:, cc, j * P128:(j + 1) * P128],
                qt_sb[:, cc, tqb * 512:(tqb + 1) * 512],
                start=(cc == 0), stop=(cc == NCC - 1))
        nc.vector.tensor_copy(
            qpt_sb[:, j, tqb * 512:(tqb + 1) * 512], qps[:, 0:512])

    attn_sb = persist.tile([P128, NTP, T], BF)   # attn_outT (normalized), bf16

    def attn_block(t, tqb):
        tq0 = tqb * TQB
        pv = ps_pv.tile([P128, 1024], F32, tag="ps_pv", name=f"pv_{t}_{tqb}")
        for c in range(NTK):
            s2 = ps_s2.tile([P128, 1024], F32, tag="ps_s2", name=f"s2_{t}_{tqb}_{c}")
            # head pair via PE row tiling (K=64 at base partitions 0 / 64)
            nc.tensor.matmul(
                s2[:, 0:512],
                k2_sb[0:64, c * P128:(c + 1) * P128],
                qpt_sb[0:64, t, tq0:tq0 + TQB],
                start=True, stop=True)
            nc.tensor.matmul(
                s2[:, 512:1024],
                k2_sb[64:128, c * P128:(c + 1) * P128],
                qpt_sb[64:128, t, tq0:tq0 + TQB],
                start=True, stop=True)
            p = ppool.tile([P128, 1024], BF, tag="p", name=f"p_{t}_{tqb}_{c}")
            nc.scalar.activation(p, s2, EXP, scale=SCALE)
            nc.tensor.matmul(
                pv[0:65, 0:512], v65_sb[:, c, :], p[:, 0:512],
                start=(c == 0), stop=(c == NTK - 1))
            nc.tensor.matmul(
                pv[0:65, 512:1024], v65_sb[:, c, :], p[:, 512:1024],
                start=(c == 0), stop=(c == NTK - 1))
        # normalize: rows 0..63 / row 64 (per tq, per head).
        # Copy PSUM->SBUF first so the pv slot frees fast (the division
        # chain has two ~2us DMAs in it; holding pv through it stalls the
        # next block's PV chain and starves ACT).
        pvs = small.tile([65, 1024], F32, tag="pvs", name=f"pvs_{t}_{tqb}")
        nc.vector.tensor_copy(pvs, pv[0:65, :])
        ss = small.tile([1, 1024], F32, tag="ss", name=f"ss_{t}_{tqb}")
        nc.vector.tensor_copy(ss, pv[64:65, :])
        r = small.tile([1, 1024], F32, tag="r", name=f"r_{t}_{tqb}")
        nc.vector.reciprocal_approx_fast(out=r, in_=ss)
        # partition-broadcast r across 64 lanes (bounce via DRAM: DMA
        # cannot zero-step an SBUF source partition)
        rd = dram.tile([1, 1024], F32, tag="rd", name=f"rd_{t}_{tqb}")
        nc.sync.dma_start(rd, r)
        rb = small.tile([64, 1024], F32, tag="rb", name=f"rb_{t}_{tqb}")
        nc.sync.dma_start(rb, rd.to_broadcast([64, 1024]))
        nc.vector.tensor_mul(
            attn_sb[0:64, t, tq0:tq0 + TQB], pvs[0:64, 0:512], rb[:, 0:512])
        h2s = small.tile([64, 512], BF, tag="h2s", name=f"h2s_{t}_{tqb}")
        nc.vector.tensor_mul(h2s, pvs[0:64, 512:1024], rb[:, 512:1024])
        nc.sync.dma_start(attn_sb[64:128, t, tq0:tq0 + TQB], h2s)

    def wp_tile(tt):
        po = ps_wp.tile([P128, 1024], F32, tag="ps_wp", name=f"po_{tt}")
        for rr in range(HD // P128):
            lhsT = attn_sb[:, rr, tt * P128:(tt + 1) * P128]
            nc.tensor.matmul(po[:, 0:512], lhsT, wp_sb[:, rr, 0:512],
                             start=(rr == 0), stop=(rr == 3))
            nc.tensor.matmul(po[:, 512:1024], lhsT, wp_sb[:, rr, 512:1024],
                             start=(rr == 0), stop=(rr == 3))
        os_ = outp.tile([P128, 1024], F32, tag="os", name=f"os_{tt}")
        nc.vector.tensor_copy(os_, po)
        nc.sync.dma_start(dr["out"].ap()[tt * P128:(tt + 1) * P128, :], os_)

    # schedule: tqb-outer so Wp(tqb) overlaps the next tqb's attention;
    # q-proj chains for tqb+1 interleave into tqb's ACT-bound stream.
    for j in range(NTP):
        qproj_chain(j, 0)
    for tqb in range(NTQB):
        for t in range(NTP):
            attn_block(t, tqb)
            if tqb < NTQB - 1:
                qproj_chain(t, tqb + 1)
        for tt in range(4 * tqb, 4 * tqb + 4):
            wp_tile(tt)


def build_nc():
    nc = bacc.Bacc("TRN2", target_bir_lowering=False, debug=False)
    dr = {
        "qT": nc.dram_tensor("qT", [C, T], BF, kind="ExternalInput"),
        "kT": nc.dram_tensor("kT", [C, T], BF, kind="ExternalInput"),
        "vT": nc.dram_tensor("vT", [C, T], BF, kind="ExternalInput"),
        "wq": nc.dram_tensor("wq", [C, HD], BF, kind="ExternalInput"),
        "wk2": nc.dram_tensor("wk2", [C, P128], BF, kind="ExternalInput"),
        "wv": nc.dram_tensor("wv", [C, D], BF, kind="ExternalInput"),
        "wp": nc.dram_tensor("wp", [HD, C], BF, kind="ExternalInput"),
        "out": nc.dram_tensor("out", [T, C], F32, kind="ExternalOutput"),
    }
    with tile.TileContext(nc) as tc, ExitStack() as ctx:
        emit_kernel(ctx, tc, dr)
    nc.compile()
    return nc


_NC_CACHE = None


def _get_nc():
    global _NC_CACHE
    if _NC_CACHE is None:
        _NC_CACHE = build_nc()
    return _NC_CACHE


def make_in_maps(q, k, v, Wq, Wk, Wv, Wp):
    """Per-core input dicts (host-side sharding + transpose + bf16 cast)."""
    bf = lambda x: np.ascontiguousarray(x).astype(NPBF)
    wk2 = np.concatenate([Wk, Wk], axis=1)
    per_b = []
    for b in range(B):
        per_b.append((bf(q[b].T), bf(k[b].T), bf(v[b].T)))
    in_maps = []
    for core in range(NCORES):
        b, g = core // 2, core % 2
        qT, kT, vT = per_b[b]
        in_maps.append({
            "qT": qT, "kT": kT, "vT": vT,
            "wq": bf(Wq[:, g * HD:(g + 1) * HD]),
            "wk2": bf(wk2),
            "wv": bf(Wv),
            "wp": bf(Wp[g * HD:(g + 1) * HD, :]),
        })
    return in_maps


def kernel(q, k, v, Wq, Wk, Wv, Wp, bp):
    from concourse.bass_utils import run_bass_kernel_spmd

    q, k, v, Wq, Wk, Wv, Wp, bp = (np.asarray(x, np.float32)
                                   for x in (q, k, v, Wq, Wk, Wv, Wp, bp))
    nc = _get_nc()
    in_maps = make_in_maps(q, k, v, Wq, Wk, Wv, Wp)
    res = run_bass_kernel_spmd(nc, in_maps, list(range(NCORES))).results
    out = np.empty((B, T, C), np.float32)
    for b in range(B):
        out[b] = res[2 * b]["out"] + res[2 * b + 1]["out"] + bp
    return out
